# revision 11
# baseline (speedup 1.0000x reference)
"""Trainium2 Bass kernel for the hinge-to-own-class-center loss.

reference:
    own = center[labels]                       # [N, D] gather
    dist = ||features - own||_2                # [N]
    loss = mean(relu(THRES - dist))            # scalar

Strategy (pure data parallel over 8 NeuronCores):
  - shard features/labels along N (8192 rows per core), replicate center
  - features are downcast to f16 on host (tolerance is 2e-2; f16 adds
    ~1e-4): halves the irreducible feature stream to 8 MiB/core
  - per core: 8 "supertiles" of 1024 rows, loaded as [128, 8, 512] with 8
    consecutive rows per partition -> 8 KiB contiguous DMA descriptors
    (>=4 KiB saturates the DMA bus; 2 KiB row descriptors measured only
    244 GB/s, this layout measured 323 GB/s)
  - center rows gathered as int8 (one global dequant scale) via SWDGE
    dma_gather, 1024 idxs per supertile, spread across all 4 SWDGE queues
    (each queue saturates one DMA engine at ~22.5 GB/s; 4 queues measured
    44 us/pass for the 4 MiB of random 512 B rows -- the pass's critical
    path). Host permutes the idx order so gathered rows land in supertile
    sample order.
  - the dequant scale is folded into the features on host (f' = f/scale,
    cscale carries scale^2 and rescales dist^2 inside the Sqrt activation),
    so the subtract is a plain tensor_sub on any engine
  - engine split, overlapped under the gather: DVE tensor_sub d = c8 - f'
    per supertile, then 8 ACT Square+accum units each (ACT accum_out is
    the only HW-reliable fused row-sum: accum_out on DVE/Pool instruction
    types silently writes nothing on HW, walrus rejects Pool stt, and
    InstTensorTensorReduce faults the exec unit; Pool tensor_sub runs far
    below its cost model, so POOL_ST stays empty)
  - epilogue: ACT sqrt -> relu(THRES - dist) with accum -> per-partition
    partial sums [128, 1]; host sums 8x128 partials / N
  - build_nc(rep=K) wraps the per-pass body in a hardware For_i loop
    running K passes per NEFF execution (used by test.py to amortize the
    ~80 ms axon-tunnel dispatch floor out of the timing; every pass redoes
    the full gather + stream + compute and rewrites the same accumulators,
    so the final output is identical). UNROLL passes per loop iteration
    amortize the For_i all-engine barrier's pipeline drain.
"""

import numpy as np

from concourse import bacc, bass, mybir
import concourse.tile as tile
from concourse.bass_utils import run_bass_kernel_spmd

N = 65536
D = 512
C = 1000
NCORES = 8
R = N // NCORES          # rows per core = 8192
P = 128                  # partitions
SPT = 8                  # samples per partition per supertile
ST = R // (P * SPT)      # supertiles per core = 8
T = R // P               # sample-groups (accum units) per core = 64
GIDX = P * SPT           # idxs per gather = 1024
THRES = 40.0

NQUEUES = 4              # ucode MAX_SWDGE_QUEUES
# Pool tensor_sub measured far slower than its cost model on HW; keep all
# subtracts on DVE and all row-sums on ACT (empty = no Pool/DVE-reduce STs)
POOL_ST = ()
UNROLL = 8               # passes per For_i iteration (rep > 1 only)

F32 = mybir.dt.float32
F16 = mybir.dt.float16
I16 = mybir.dt.int16
I8 = mybir.dt.int8


def build_nc(rep: int = 1) -> bass.Bass:
    nc = bacc.Bacc(None, target_bir_lowering=False, num_swdge_queues=NQUEUES)

    # [1024, 8, 512]: slicing 128 rows of dim0 gives a supertile AP whose
    # per-partition payload is 8 consecutive feature rows (8 KiB contiguous).
    feat = nc.declare_dram_parameter(
        "features", [R // SPT, SPT, D], F16, isOutput=False
    )
    center = nc.declare_dram_parameter("center_q", [C, D], I8, isOutput=False)
    cscale = nc.declare_dram_parameter("cscale", [P, 1], F32, isOutput=False)
    # idx i of the shard lives at [i % 16, i // 16], replicated 8x down the
    # partition dim -> [128, R // 16]; i runs in gather order (host permutes)
    idx = nc.declare_dram_parameter("idx", [P, R // 16], I16, isOutput=False)
    out = nc.declare_dram_parameter("partial", [P, 1], F32, isOutput=True)

    with tile.TileContext(nc) as tc:
        with (
            tc.tile_pool(name="fpool", bufs=3) as fpool,
            tc.tile_pool(name="cpool", bufs=3) as cpool,
            tc.tile_pool(name="dpool", bufs=3) as dpool,
            tc.tile_pool(name="sqa", bufs=2) as sqa,
            tc.tile_pool(name="sqd", bufs=2) as sqd,
            tc.tile_pool(name="acc", bufs=1) as acc,
        ):
            idx_sb = acc.tile([P, R // 16], I16)
            nc.sync.dma_start(out=idx_sb[:], in_=idx[:])

            thres_col = acc.tile([P, 1], F32)
            nc.gpsimd.memset(thres_col[:], THRES)

            scale_col = acc.tile([P, 1], F32)
            nc.sync.dma_start(out=scale_col[:], in_=cscale[:])

            dist2_all = acc.tile([P, T], F32)
            dist_all = acc.tile([P, T], F32)
            hinge_all = acc.tile([P, T], F32)
            partial = acc.tile([P, 1], F32)

            def one_pass():
                for st in range(ST):
                    c_st = cpool.tile([P, SPT, D], I8, tag="c")
                    nc.gpsimd.dma_gather(
                        out_ap=c_st[:],
                        in_ap=center[:],
                        idxs_ap=idx_sb[
                            :, st * (GIDX // 16):(st + 1) * (GIDX // 16)
                        ],
                        num_idxs=GIDX,
                        num_idxs_reg=GIDX,
                        elem_size=D,
                        queue_num=st % NQUEUES,
                    )
                    f_st = fpool.tile([P, SPT, D], F16, tag="f")
                    nc.sync.dma_start(
                        out=f_st[:], in_=feat[st * P:(st + 1) * P, :, :]
                    )
                    d_st = dpool.tile([P, SPT, D], F16, tag="d")
                    # host pre-divides features by the center dequant scale,
                    # so d = c8 - f' needs no dequant multiply; the final
                    # Sqrt activation rescales dist^2 by scale^2 (cscale)
                    sub_engine = nc.gpsimd if st in POOL_ST else nc.vector
                    sub_engine.tensor_sub(d_st[:], c_st[:], f_st[:])
                    if st in POOL_ST:
                        # DVE: square whole supertile (2x f16), then one
                        # segmented row-sum into 8 dist2 columns
                        sq_st = sqd.tile([P, SPT, D], F16, tag="sq")
                        nc.vector.tensor_mul(
                            out=sq_st[:], in0=d_st[:], in1=d_st[:]
                        )
                        nc.vector.tensor_reduce(
                            out=dist2_all[:, st * SPT:(st + 1) * SPT],
                            in_=sq_st[:],
                            axis=mybir.AxisListType.X,
                            op=mybir.AluOpType.add,
                        )
                    else:
                        for r in range(SPT):
                            t = st * SPT + r
                            sq_t = sqa.tile([P, D], F16, tag="sq")
                            nc.scalar.activation(
                                out=sq_t[:],
                                in_=d_st[:, r, :],
                                func=mybir.ActivationFunctionType.Square,
                                accum_out=dist2_all[:, t:t + 1],
                            )

                nc.scalar.activation(
                    out=dist_all[:],
                    in_=dist2_all[:],
                    func=mybir.ActivationFunctionType.Sqrt,
                    scale=scale_col[:],
                )
                nc.scalar.activation(
                    out=hinge_all[:],
                    in_=dist_all[:],
                    func=mybir.ActivationFunctionType.Relu,
                    scale=-1.0,
                    bias=thres_col[:],
                    accum_out=partial[:],
                )

            if rep == 1:
                one_pass()
            else:
                assert rep % UNROLL == 0, (rep, UNROLL)
                with tc.For_i(0, rep // UNROLL):
                    for _ in range(UNROLL):
                        one_pass()

            nc.sync.dma_start(out=out[:], in_=partial[:])

    return nc


def make_in_maps(features: np.ndarray, center: np.ndarray, labels: np.ndarray):
    feats = np.asarray(features, dtype=np.float32)
    cent = np.ascontiguousarray(np.asarray(center, dtype=np.float32))
    lab = np.asarray(labels).astype(np.int64)
    assert feats.shape == (N, D) and cent.shape == (C, D) and lab.shape == (N,)
    scale = float(np.abs(cent).max()) / 127.0
    if scale == 0.0:
        scale = 1.0
    cent_q = np.ascontiguousarray(
        np.clip(np.rint(cent / scale), -127, 127).astype(np.int8)
    )
    # cscale carries scale^2: the kernel computes d = c8 - f/scale, so
    # dist = sqrt(sum(d^2) * scale^2) via the Sqrt activation's scale input
    scale_col = np.full((P, 1), scale * scale, dtype=np.float32)
    feats16 = (feats * (1.0 / scale)).astype(np.float16)

    in_maps = []
    for c in range(NCORES):
        sl = slice(c * R, (c + 1) * R)
        # gather order: idx[st*1024 + j*128 + p] = labels[st*1024 + 8p + j]
        # so gathered row (p, j) matches feature sample 8p + j of supertile st
        lab_shard = lab[sl].astype(np.int16)
        perm = (
            lab_shard.reshape(ST, P, SPT).transpose(0, 2, 1).reshape(R)
        )
        wrapped = perm.reshape(R // 16, 16).T          # [16, R // 16]
        idx_full = np.ascontiguousarray(np.tile(wrapped, (P // 16, 1)))
        in_maps.append(
            {
                "features": np.ascontiguousarray(
                    feats16[sl].reshape(R // SPT, SPT, D)
                ),
                "center_q": cent_q,
                "cscale": scale_col,
                "idx": idx_full,
            }
        )
    return in_maps


_NC_CACHE = {}


def kernel(features, center, labels) -> np.ndarray:
    if "nc" not in _NC_CACHE:
        nc = build_nc()
        nc.finalize()
        _NC_CACHE["nc"] = nc
    nc = _NC_CACHE["nc"]
    in_maps = make_in_maps(features, center, labels)
    res = run_bass_kernel_spmd(nc, in_maps, list(range(NCORES)))
    total = 0.0
    for r in res.results:
        total += float(r["partial"].astype(np.float64).sum())
    return np.array(total / N, dtype=np.float32)


# revision 12
# speedup vs baseline: 1.3694x; 1.3694x over previous
"""Trainium2 Bass kernel for the hinge-to-own-class-center loss.

reference:
    own = center[labels]                       # [N, D] gather
    dist = ||features - own||_2                # [N]
    loss = mean(relu(THRES - dist))            # scalar

Strategy (pure data parallel over 8 NeuronCores):
  - shard features/labels along N (8192 rows per core), replicate center
  - features are downcast to f16 on host (tolerance is 2e-2; f16 adds
    ~1e-4): halves the irreducible feature stream to 8 MiB/core
  - per core: 8 "supertiles" of 1024 rows, loaded as [128, 8, 512] with 8
    consecutive rows per partition -> 8 KiB contiguous DMA descriptors
    (>=4 KiB saturates the DMA bus; 2 KiB row descriptors measured only
    244 GB/s, this layout measured 323 GB/s)
  - center rows gathered as int8 (one global dequant scale) via SWDGE
    dma_gather, 1024 idxs per supertile, spread across all 4 SWDGE queues
    (each queue saturates one DMA engine at ~22.5 GB/s; 4 queues measured
    44 us/pass for the 4 MiB of random 512 B rows -- the pass's critical
    path). Host permutes the idx order so gathered rows land in supertile
    sample order.
  - the dequant scale is folded into the features on host (f' = f/scale,
    cscale carries scale^2 and rescales dist^2 inside the Sqrt activation),
    so the subtract is a plain tensor_sub on any engine
  - engine split, overlapped under the gather: DVE tensor_sub d = c8 - f'
    per supertile, then 8 ACT Square+accum units each (ACT accum_out is
    the only HW-reliable fused row-sum: accum_out on DVE/Pool instruction
    types silently writes nothing on HW, walrus rejects Pool stt, and
    InstTensorTensorReduce faults the exec unit; Pool tensor_sub runs far
    below its cost model, so POOL_ST stays empty)
  - epilogue: ACT sqrt -> relu(THRES - dist) with accum -> per-partition
    partial sums [128, 1]; host sums 8x128 partials / N
  - build_nc(rep=K) wraps the per-pass body in a hardware For_i loop
    running K passes per NEFF execution (used by test.py to amortize the
    ~80 ms axon-tunnel dispatch floor out of the timing; every pass redoes
    the full gather + stream + compute and rewrites the same accumulators,
    so the final output is identical). UNROLL passes per loop iteration
    amortize the For_i all-engine barrier's pipeline drain.
"""

import numpy as np

from concourse import bacc, bass, mybir
import concourse.tile as tile
from concourse.bass_utils import run_bass_kernel_spmd

N = 65536
D = 512
C = 1000
NCORES = 8
R = N // NCORES          # rows per core = 8192
P = 128                  # partitions
SPT = 8                  # samples per partition per supertile
ST = R // (P * SPT)      # supertiles per core = 8
T = R // P               # sample-groups (accum units) per core = 64
GIDX = P * SPT           # idxs per gather = 1024
THRES = 40.0

NQUEUES = 4              # ucode MAX_SWDGE_QUEUES
# Pool tensor_sub measured far slower than its cost model on HW; all
# subtracts stay on DVE (POOL_ST empty)
POOL_ST = ()
# supertiles whose row-sums run on DVE (tensor_mul + segmented
# tensor_reduce) instead of ACT Square+accum: rebalances ACT ~54us ->
# ~41us busy vs DVE ~34 -> ~47us, under the ~44us gather wall
DVE_REDUCE_ST = (3, 7)
UNROLL = 8               # passes per For_i iteration (rep > 1 only)

F32 = mybir.dt.float32
F16 = mybir.dt.float16
I16 = mybir.dt.int16
I8 = mybir.dt.int8


def build_nc(rep: int = 1) -> bass.Bass:
    nc = bacc.Bacc(None, target_bir_lowering=False, num_swdge_queues=NQUEUES)

    # [1024, 8, 512]: slicing 128 rows of dim0 gives a supertile AP whose
    # per-partition payload is 8 consecutive feature rows (8 KiB contiguous).
    feat = nc.declare_dram_parameter(
        "features", [R // SPT, SPT, D], F16, isOutput=False
    )
    center = nc.declare_dram_parameter("center_q", [C, D], I8, isOutput=False)
    cscale = nc.declare_dram_parameter("cscale", [P, 1], F32, isOutput=False)
    # idx i of the shard lives at [i % 16, i // 16], replicated 8x down the
    # partition dim -> [128, R // 16]; i runs in gather order (host permutes)
    idx = nc.declare_dram_parameter("idx", [P, R // 16], I16, isOutput=False)
    out = nc.declare_dram_parameter("partial", [P, 1], F32, isOutput=True)

    with tile.TileContext(nc) as tc:
        with (
            tc.tile_pool(name="fpool", bufs=4) as fpool,
            tc.tile_pool(name="cpool", bufs=4) as cpool,
            tc.tile_pool(name="dpool", bufs=4) as dpool,
            tc.tile_pool(name="sqa", bufs=3) as sqa,
            tc.tile_pool(name="sqd", bufs=2) as sqd,
            tc.tile_pool(name="acc", bufs=1) as acc,
        ):
            idx_sb = acc.tile([P, R // 16], I16)
            nc.sync.dma_start(out=idx_sb[:], in_=idx[:])

            thres_col = acc.tile([P, 1], F32)
            nc.gpsimd.memset(thres_col[:], THRES)

            scale_col = acc.tile([P, 1], F32)
            nc.sync.dma_start(out=scale_col[:], in_=cscale[:])

            dist2_all = acc.tile([P, T], F32)
            dist_all = acc.tile([P, T], F32)
            hinge_all = acc.tile([P, T], F32)
            partial = acc.tile([P, 1], F32)

            def one_pass():
                for st in range(ST):
                    c_st = cpool.tile([P, SPT, D], I8, tag="c")
                    nc.gpsimd.dma_gather(
                        out_ap=c_st[:],
                        in_ap=center[:],
                        idxs_ap=idx_sb[
                            :, st * (GIDX // 16):(st + 1) * (GIDX // 16)
                        ],
                        num_idxs=GIDX,
                        num_idxs_reg=GIDX,
                        elem_size=D,
                        queue_num=st % NQUEUES,
                    )
                    f_st = fpool.tile([P, SPT, D], F16, tag="f")
                    nc.sync.dma_start(
                        out=f_st[:], in_=feat[st * P:(st + 1) * P, :, :]
                    )
                    d_st = dpool.tile([P, SPT, D], F16, tag="d")
                    # host pre-divides features by the center dequant scale,
                    # so d = c8 - f' needs no dequant multiply; the final
                    # Sqrt activation rescales dist^2 by scale^2 (cscale)
                    sub_engine = nc.gpsimd if st in POOL_ST else nc.vector
                    sub_engine.tensor_sub(d_st[:], c_st[:], f_st[:])
                    if st in DVE_REDUCE_ST:
                        # DVE: square whole supertile (2x f16), then one
                        # segmented row-sum into 8 dist2 columns
                        sq_st = sqd.tile([P, SPT, D], F16, tag="sq")
                        nc.vector.tensor_mul(
                            out=sq_st[:], in0=d_st[:], in1=d_st[:]
                        )
                        nc.vector.tensor_reduce(
                            out=dist2_all[:, st * SPT:(st + 1) * SPT],
                            in_=sq_st[:],
                            axis=mybir.AxisListType.X,
                            op=mybir.AluOpType.add,
                        )
                    else:
                        for r in range(SPT):
                            t = st * SPT + r
                            sq_t = sqa.tile([P, D], F16, tag="sq")
                            nc.scalar.activation(
                                out=sq_t[:],
                                in_=d_st[:, r, :],
                                func=mybir.ActivationFunctionType.Square,
                                accum_out=dist2_all[:, t:t + 1],
                            )

                nc.scalar.activation(
                    out=dist_all[:],
                    in_=dist2_all[:],
                    func=mybir.ActivationFunctionType.Sqrt,
                    scale=scale_col[:],
                )
                nc.scalar.activation(
                    out=hinge_all[:],
                    in_=dist_all[:],
                    func=mybir.ActivationFunctionType.Relu,
                    scale=-1.0,
                    bias=thres_col[:],
                    accum_out=partial[:],
                )

            if rep == 1:
                one_pass()
            else:
                assert rep % UNROLL == 0, (rep, UNROLL)
                with tc.For_i(0, rep // UNROLL):
                    for _ in range(UNROLL):
                        one_pass()

            nc.sync.dma_start(out=out[:], in_=partial[:])

    return nc


def make_in_maps(features: np.ndarray, center: np.ndarray, labels: np.ndarray):
    feats = np.asarray(features, dtype=np.float32)
    cent = np.ascontiguousarray(np.asarray(center, dtype=np.float32))
    lab = np.asarray(labels).astype(np.int64)
    assert feats.shape == (N, D) and cent.shape == (C, D) and lab.shape == (N,)
    scale = float(np.abs(cent).max()) / 127.0
    if scale == 0.0:
        scale = 1.0
    cent_q = np.ascontiguousarray(
        np.clip(np.rint(cent / scale), -127, 127).astype(np.int8)
    )
    # cscale carries scale^2: the kernel computes d = c8 - f/scale, so
    # dist = sqrt(sum(d^2) * scale^2) via the Sqrt activation's scale input
    scale_col = np.full((P, 1), scale * scale, dtype=np.float32)
    feats16 = (feats * (1.0 / scale)).astype(np.float16)

    in_maps = []
    for c in range(NCORES):
        sl = slice(c * R, (c + 1) * R)
        # gather order: idx[st*1024 + j*128 + p] = labels[st*1024 + 8p + j]
        # so gathered row (p, j) matches feature sample 8p + j of supertile st
        lab_shard = lab[sl].astype(np.int16)
        perm = (
            lab_shard.reshape(ST, P, SPT).transpose(0, 2, 1).reshape(R)
        )
        wrapped = perm.reshape(R // 16, 16).T          # [16, R // 16]
        idx_full = np.ascontiguousarray(np.tile(wrapped, (P // 16, 1)))
        in_maps.append(
            {
                "features": np.ascontiguousarray(
                    feats16[sl].reshape(R // SPT, SPT, D)
                ),
                "center_q": cent_q,
                "cscale": scale_col,
                "idx": idx_full,
            }
        )
    return in_maps


_NC_CACHE = {}


def kernel(features, center, labels) -> np.ndarray:
    if "nc" not in _NC_CACHE:
        nc = build_nc()
        nc.finalize()
        _NC_CACHE["nc"] = nc
    nc = _NC_CACHE["nc"]
    in_maps = make_in_maps(features, center, labels)
    res = run_bass_kernel_spmd(nc, in_maps, list(range(NCORES)))
    total = 0.0
    for r in res.results:
        total += float(r["partial"].astype(np.float64).sum())
    return np.array(total / N, dtype=np.float32)
